# revision 3
# baseline (speedup 1.0000x reference)
"""Trainium2 Bass kernel for HDGradientCompressionLayer forward.

Reference computation: y = einsum("bsd,df->bsf", x, W) + b
  x: (4, 4096, 1024) f32, W: (1024, 1024) f32, b: (1024,) f32.

Strategy (data-parallel across 8 cores, per sharding hint):
  Flatten x to (16384, 1024); each core gets 2048 rows. Per core the
  kernel computes y_shard = x_shard @ W + b:
    - x rowblock [128, 1024] is cast-loaded f32->bf16 on the SWDGE
      queue (gpsimd), which carries nothing else so rowblocks arrive
      every ~1.3us,
    - the sync-engine xbar DMA transpose turns each rowblock into the
      8 stationary [d,row] tiles (out[p,k,j] = in[j, k*128+p]) --
      keeping the PE queue free of transpose work,
    - W is loaded f32 on the scalar HWDGE queue and cast to bf16 on
      DVE per k-block,
    - rowblocks 0-3 run k-outer across all 8 PSUM banks so the PE has
      work while W streams in (this phase also absorbs the PE clock
      ramp); rowblocks 4-15 then stream 16 bf16 matmuls each,
    - DVE adds the (partition-broadcast) f32 bias during PSUM->SBUF
      eviction, scalar HWDGE stores f32 y rowblocks (4KB descriptors).
"""

import os
from contextlib import ExitStack

import numpy as np

import concourse.bass as bass
import concourse.bacc as bacc
import concourse.tile as tile
from concourse import mybir
from concourse.bass_utils import run_bass_kernel_spmd

N_CORES = 8
B, S, D = 4, 4096, 1024
F = 1024
ROWS_TOTAL = B * S          # 16384
ROWS = ROWS_TOTAL // N_CORES  # 2048 per core
P = 128
NSPLIT = 512                # one PSUM bank of f32
KB = D // P                 # 8 contraction blocks
NB = F // NSPLIT            # 2 psum banks per rowblock
GROUP = 4                   # rowblocks in the k-outer warm phase


def build_nc(rows: int = ROWS) -> bass.Bass:
    nc = bacc.Bacc("TRN2", target_bir_lowering=False, debug=False)
    x = nc.dram_tensor("x", [rows, D], mybir.dt.float32, kind="ExternalInput").ap()
    W = nc.dram_tensor("W", [D, F], mybir.dt.float32, kind="ExternalInput").ap()
    b = nc.dram_tensor("b", [F], mybir.dt.float32, kind="ExternalInput").ap()
    y = nc.dram_tensor("y", [rows, F], mybir.dt.float32, kind="ExternalOutput").ap()

    RB = rows // P     # rowblocks

    with tile.TileContext(nc) as tc, ExitStack() as ctx:
        const = ctx.enter_context(tc.tile_pool(name="const", bufs=1))
        xp = ctx.enter_context(tc.tile_pool(name="xp", bufs=RB))
        xtp = ctx.enter_context(tc.tile_pool(name="xtp", bufs=RB))
        yp = ctx.enter_context(tc.tile_pool(name="yp", bufs=4))
        psp = ctx.enter_context(tc.tile_pool(name="psp", bufs=1, space="PSUM"))

        # W: f32 staging (scalar HWDGE load) + bf16 (DVE cast), laid out
        # [p, k, f] with d = k*128 + p to match the xbar-transpose output.
        W_f32 = const.tile([P, KB, F], mybir.dt.float32)
        W_bf = const.tile([P, KB, F], mybir.dt.bfloat16)
        W_pkf = W.rearrange("(k p) f -> p k f", p=P)

        # Bias broadcast to all partitions, f32.
        b_bc = const.tile([P, F], mybir.dt.float32)

        # Zeroed tile for PE clock-ramp warmup.
        warm = const.tile([P, P], mybir.dt.bfloat16)
        nc.any.memset(warm[:], 0.0)

        def ps_tile():
            return psp.tile([P, NSPLIT], mybir.dt.float32, name="ps", tag="ps", bufs=8)

        # SWDGE: x cast-loads only, so rowblocks arrive back-to-back.
        x_tiles = []
        for rb in range(RB):
            x_bf = xp.tile([P, D], mybir.dt.bfloat16, name="x_bf", tag="x_bf")
            nc.gpsimd.dma_start(x_bf[:], x[rb * P:(rb + 1) * P, :])  # cast load
            x_tiles.append(x_bf)

        # Scalar HWDGE: W f32 per k-block (y stores follow later in
        # program order on the same queue).
        for k in range(KB):
            nc.scalar.dma_start(W_f32[:, k, :], W_pkf[:, k, :])

        # Sync HWDGE: xbar transposes (bias broadcast interleaved early).
        xT_tiles = []
        for rb in range(RB):
            xT = xtp.tile([P, KB, P], mybir.dt.bfloat16, name="xT", tag="xT")
            nc.sync.dma_start_transpose(xT[:], x_tiles[rb][:])
            xT_tiles.append(xT)
            if rb == 1:
                nc.sync.dma_start(
                    b_bc[:], b.rearrange("(o f) -> o f", o=1).to_broadcast([P, F])
                )

        # DVE: cast W to bf16 per k-block as it lands.
        for k in range(KB):
            nc.vector.tensor_copy(W_bf[:, k, :], W_f32[:, k, :])

        # PE warmup: short matmuls ramp the clock while the first x
        # rowblock and W k-block land.
        warm_ps = ps_tile()
        for _ in range(14):
            nc.tensor.matmul(
                warm_ps[:, 0:P], warm[:], warm[:, 0:1].to_broadcast([P, P]),
                start=True, stop=True, skip_group_check=True,
            )

        def evict(rb: int, pss) -> None:
            y_sb = yp.tile([P, F], mybir.dt.float32)
            for n in range(NB):
                nc.vector.tensor_add(
                    y_sb[:, n * NSPLIT:(n + 1) * NSPLIT],
                    pss[n][:],
                    b_bc[:, n * NSPLIT:(n + 1) * NSPLIT],
                )
            nc.scalar.dma_start(y[rb * P:(rb + 1) * P, :], y_sb[:])

        # Phase 1 - rowblocks 0..GROUP-1, k-outer across all 8 banks so
        # the PE can follow W's arrival k-block by k-block.
        psA = [[ps_tile() for _ in range(NB)] for _ in range(GROUP)]
        for k in range(KB):
            for r in range(GROUP):
                for n in range(NB):
                    nc.tensor.matmul(
                        psA[r][n][:],
                        xT_tiles[r][:, k, :],
                        W_bf[:, k, n * NSPLIT:(n + 1) * NSPLIT],
                        start=(k == 0),
                        stop=(k == KB - 1),
                    )
        for r in range(GROUP):
            evict(r, psA[r])

        # Phase 2 - rowblocks GROUP..RB-1 stream with k-inner.
        for rb in range(GROUP, RB):
            pss = [ps_tile() for _ in range(NB)]
            for k in range(KB):
                for n in range(NB):
                    nc.tensor.matmul(
                        pss[n][:],
                        xT_tiles[rb][:, k, :],
                        W_bf[:, k, n * NSPLIT:(n + 1) * NSPLIT],
                        start=(k == 0),
                        stop=(k == KB - 1),
                    )
            evict(rb, pss)

    nc.compile()
    return nc


_NC_CACHE: dict[int, bass.Bass] = {}


def _get_nc(rows: int = ROWS) -> bass.Bass:
    if rows not in _NC_CACHE:
        _NC_CACHE[rows] = build_nc(rows)
    return _NC_CACHE[rows]


def _run(in_maps, rows: int = ROWS, trace: bool = False):
    nc = _get_nc(rows)
    return run_bass_kernel_spmd(nc, in_maps, list(range(N_CORES)), trace=trace)


def kernel(x: np.ndarray, W: np.ndarray, b: np.ndarray) -> np.ndarray:
    x = np.ascontiguousarray(np.asarray(x, dtype=np.float32))
    W = np.ascontiguousarray(np.asarray(W, dtype=np.float32))
    b = np.ascontiguousarray(np.asarray(b, dtype=np.float32))
    x_flat = x.reshape(ROWS_TOTAL, D)
    in_maps = [
        {"x": np.ascontiguousarray(x_flat[c * ROWS:(c + 1) * ROWS]), "W": W, "b": b}
        for c in range(N_CORES)
    ]
    res = _run(in_maps, trace=bool(int(os.environ.get("BASS_KERNEL_TRACE", "0"))))
    y = np.concatenate([res.results[c]["y"] for c in range(N_CORES)], axis=0)
    return y.reshape(B, S, F)


# revision 4
# speedup vs baseline: 1.0134x; 1.0134x over previous
"""Trainium2 Bass kernel for HDGradientCompressionLayer forward.

Reference computation: y = einsum("bsd,df->bsf", x, W) + b
  x: (4, 4096, 1024) f32, W: (1024, 1024) f32, b: (1024,) f32.

Strategy (data-parallel across 8 cores, per sharding hint):
  Flatten x to (16384, 1024); each core gets 2048 rows. Per core the
  kernel computes y_shard = x_shard @ W + b:
    - x rowblocks [128, 1024] are cast-loaded f32->bf16 on the SWDGE
      queue (gpsimd), which carries nothing else so rowblocks arrive
      every ~1.3us,
    - W is loaded f32 on the scalar HWDGE queue (4 chunks) and cast to
      bf16 on DVE, so it does not contend with x for the SWDGE queue,
    - per rowblock the PE transposes the 8 [128,128] x tiles into PSUM
      (~0.6us burst), scalar evicts them to SBUF, then 16 bf16 matmuls
      (N=512, PSUM-accumulated over the 8 d-blocks),
    - rowblocks 0-2 run k-outer across 6 PSUM banks so the PE can
      follow W's chunk-by-chunk arrival (this also absorbs the PE
      clock ramp); rowblocks 3-15 then stream at 216ns/matmul,
    - DVE adds the (partition-broadcast) f32 bias during PSUM->SBUF
      eviction, scalar HWDGE stores f32 y rowblocks (4KB descriptors).
"""

import os
from contextlib import ExitStack

import numpy as np

import concourse.bass as bass
import concourse.bacc as bacc
import concourse.tile as tile
from concourse import mybir
from concourse.bass_utils import run_bass_kernel_spmd
from concourse.masks import make_identity

N_CORES = 8
B, S, D = 4, 4096, 1024
F = 1024
ROWS_TOTAL = B * S          # 16384
ROWS = ROWS_TOTAL // N_CORES  # 2048 per core
P = 128
NSPLIT = 512                # one PSUM bank of f32
KB = D // P                 # 8 contraction blocks
NB = F // NSPLIT            # 2 psum banks per rowblock
GROUP = 3                   # rowblocks in the k-outer warm phase
WCHUNK = 2                  # W k-blocks per HWDGE load chunk


def build_nc(rows: int = ROWS) -> bass.Bass:
    nc = bacc.Bacc("TRN2", target_bir_lowering=False, debug=False)
    x = nc.dram_tensor("x", [rows, D], mybir.dt.float32, kind="ExternalInput").ap()
    W = nc.dram_tensor("W", [D, F], mybir.dt.float32, kind="ExternalInput").ap()
    b = nc.dram_tensor("b", [F], mybir.dt.float32, kind="ExternalInput").ap()
    y = nc.dram_tensor("y", [rows, F], mybir.dt.float32, kind="ExternalOutput").ap()

    RB = rows // P     # rowblocks

    with tile.TileContext(nc) as tc, ExitStack() as ctx:
        const = ctx.enter_context(tc.tile_pool(name="const", bufs=1))
        xp = ctx.enter_context(tc.tile_pool(name="xp", bufs=RB))
        xtp = ctx.enter_context(tc.tile_pool(name="xtp", bufs=RB))
        yp = ctx.enter_context(tc.tile_pool(name="yp", bufs=4))
        psp = ctx.enter_context(tc.tile_pool(name="psp", bufs=1, space="PSUM"))

        # W: f32 staging (scalar HWDGE load) + bf16 (DVE cast), laid out
        # [p, k, f] with d = k*128 + p to match the PE-transpose layout.
        W_f32 = const.tile([P, KB, F], mybir.dt.float32)
        W_bf = const.tile([P, KB, F], mybir.dt.bfloat16)
        W_pkf = W.rearrange("(k p) f -> p k f", p=P)

        # Bias broadcast to all partitions, f32 (sync HWDGE queue).
        b_bc = const.tile([P, F], mybir.dt.float32)

        # Identity for PE-based transposes; zeroed tile for clock warmup.
        ident = const.tile([P, P], mybir.dt.bfloat16)
        make_identity(nc, ident[:])
        warm = const.tile([P, P], mybir.dt.bfloat16)
        nc.any.memset(warm[:], 0.0)

        def ps_tile():
            return psp.tile([P, NSPLIT], mybir.dt.float32, name="ps", tag="ps", bufs=2 * GROUP)

        # SWDGE: x cast-loads only, so rowblocks arrive back-to-back.
        x_tiles = []
        for rb in range(RB):
            x_bf = xp.tile([P, D], mybir.dt.bfloat16, name="x_bf", tag="x_bf")
            nc.gpsimd.dma_start(x_bf[:], x[rb * P:(rb + 1) * P, :])  # cast load
            x_tiles.append(x_bf)

        # Scalar HWDGE: W f32 chunks (y stores follow later in program
        # order on the same queue).
        for c in range(KB // WCHUNK):
            k0, k1 = c * WCHUNK, (c + 1) * WCHUNK
            nc.scalar.dma_start(W_f32[:, k0:k1, :], W_pkf[:, k0:k1, :])

        # Sync HWDGE: bias broadcast.
        nc.sync.dma_start(
            b_bc[:], b.rearrange("(o f) -> o f", o=1).to_broadcast([P, F])
        )

        # DVE: cast W chunks to bf16 as they land (evictions queue after).
        for c in range(KB // WCHUNK):
            k0, k1 = c * WCHUNK, (c + 1) * WCHUNK
            nc.vector.tensor_copy(W_bf[:, k0:k1, :], W_f32[:, k0:k1, :])

        def warmup(n):
            for _ in range(n):
                nc.tensor.matmul(
                    warm_ps[:, 0:P], warm[:], warm[:, 0:1].to_broadcast([P, P]),
                    start=True, stop=True, skip_group_check=True,
                )

        def transpose(rb: int):
            # PE transposes the 8 k-tiles into one PSUM bank; scalar
            # copies them back to SBUF as the stationary layout.
            psT = psp.tile([P, KB, P], mybir.dt.bfloat16, name="psT", tag="psT", bufs=2)
            for k in range(KB):
                nc.tensor.transpose(psT[:, k, :], x_tiles[rb][:, k * P:(k + 1) * P], ident[:])
            xT = xtp.tile([P, KB, P], mybir.dt.bfloat16, name="xT", tag="xT")
            nc.scalar.copy(xT[:], psT[:])
            return xT

        def evict(rb: int, pss) -> None:
            y_sb = yp.tile([P, F], mybir.dt.float32)
            for n in range(NB):
                nc.vector.tensor_add(
                    y_sb[:, n * NSPLIT:(n + 1) * NSPLIT],
                    pss[n][:],
                    b_bc[:, n * NSPLIT:(n + 1) * NSPLIT],
                )
            nc.scalar.dma_start(y[rb * P:(rb + 1) * P, :], y_sb[:])

        # PE warmup ramps the clock while the first x rowblock lands.
        warm_ps = ps_tile()
        warmup(12)

        # Phase 1 - rowblocks 0..GROUP-1: transposes as x arrives
        # (warmup matmuls fill the gaps), then k-outer across 6 banks
        # following W's chunk arrivals.
        xT_tiles = {}
        for r in range(GROUP):
            xT_tiles[r] = transpose(r)
            if r < GROUP - 1:
                warmup(6)
        psA = [[ps_tile() for _ in range(NB)] for _ in range(GROUP)]
        for k in range(KB):
            for r in range(GROUP):
                for n in range(NB):
                    nc.tensor.matmul(
                        psA[r][n][:],
                        xT_tiles[r][:, k, :],
                        W_bf[:, k, n * NSPLIT:(n + 1) * NSPLIT],
                        start=(k == 0),
                        stop=(k == KB - 1),
                    )
        for r in range(GROUP):
            evict(r, psA[r])

        # Phase 2 - rowblocks GROUP..RB-1 stream with k-inner.
        for rb in range(GROUP, RB):
            xT = transpose(rb)
            pss = [ps_tile() for _ in range(NB)]
            for k in range(KB):
                for n in range(NB):
                    nc.tensor.matmul(
                        pss[n][:],
                        xT[:, k, :],
                        W_bf[:, k, n * NSPLIT:(n + 1) * NSPLIT],
                        start=(k == 0),
                        stop=(k == KB - 1),
                    )
            evict(rb, pss)

    nc.compile()
    return nc


_NC_CACHE: dict[int, bass.Bass] = {}


def _get_nc(rows: int = ROWS) -> bass.Bass:
    if rows not in _NC_CACHE:
        _NC_CACHE[rows] = build_nc(rows)
    return _NC_CACHE[rows]


def _run(in_maps, rows: int = ROWS, trace: bool = False):
    nc = _get_nc(rows)
    return run_bass_kernel_spmd(nc, in_maps, list(range(N_CORES)), trace=trace)


def kernel(x: np.ndarray, W: np.ndarray, b: np.ndarray) -> np.ndarray:
    x = np.ascontiguousarray(np.asarray(x, dtype=np.float32))
    W = np.ascontiguousarray(np.asarray(W, dtype=np.float32))
    b = np.ascontiguousarray(np.asarray(b, dtype=np.float32))
    x_flat = x.reshape(ROWS_TOTAL, D)
    in_maps = [
        {"x": np.ascontiguousarray(x_flat[c * ROWS:(c + 1) * ROWS]), "W": W, "b": b}
        for c in range(N_CORES)
    ]
    res = _run(in_maps, trace=bool(int(os.environ.get("BASS_KERNEL_TRACE", "0"))))
    y = np.concatenate([res.results[c]["y"] for c in range(N_CORES)], axis=0)
    return y.reshape(B, S, F)


# revision 5
# speedup vs baseline: 1.2539x; 1.2373x over previous
"""Trainium2 Bass kernel for HDGradientCompressionLayer forward.

Reference computation: y = einsum("bsd,df->bsf", x, W) + b
  x: (4, 4096, 1024) f32, W: (1024, 1024) f32, b: (1024,) f32.

Strategy (data-parallel across 8 cores, per sharding hint):
  Flatten x to (16384, 1024); each core gets 2048 rows. Per core the
  kernel computes y_shard = x_shard @ W + b:
    - x rowblocks and W k-blocks are cast-loaded f32->bf16 on the
      SWDGE queue, interleaved so x0-x2 and W land early,
    - per rowblock the PE transposes the 8 [128,128] x tiles into PSUM
      (~0.6us burst) and scalar/vector alternate evicting them to SBUF,
    - rowblocks 0-2 run k-outer across 6 PSUM banks so the PE can
      follow W's k-block-by-k-block arrival (this phase also absorbs
      the PE clock ramp; warmup matmuls fill the leading gaps),
    - rowblocks 3-15 then stream 16 bf16 matmuls each (N=512,
      PSUM-accumulated over the 8 d-blocks) at full PE rate,
    - DVE adds the (partition-broadcast) f32 bias during PSUM->SBUF
      eviction, scalar HWDGE stores f32 y rowblocks (4KB descriptors).
"""

import os
from contextlib import ExitStack

import numpy as np

import concourse.bass as bass
import concourse.bacc as bacc
import concourse.tile as tile
from concourse import mybir
from concourse.bass_utils import run_bass_kernel_spmd
from concourse.masks import make_identity

N_CORES = 8
B, S, D = 4, 4096, 1024
F = 1024
ROWS_TOTAL = B * S          # 16384
ROWS = ROWS_TOTAL // N_CORES  # 2048 per core
P = 128
NSPLIT = 512                # one PSUM bank of f32
KB = D // P                 # 8 contraction blocks
NB = F // NSPLIT            # 2 psum banks per rowblock
GROUP = 3                   # rowblocks in the k-outer warm phase


def build_nc(rows: int = ROWS) -> bass.Bass:
    nc = bacc.Bacc("TRN2", target_bir_lowering=False, debug=False)
    x = nc.dram_tensor("x", [rows, D], mybir.dt.float32, kind="ExternalInput").ap()
    W = nc.dram_tensor("W", [D, F], mybir.dt.float32, kind="ExternalInput").ap()
    b = nc.dram_tensor("b", [F], mybir.dt.float32, kind="ExternalInput").ap()
    y = nc.dram_tensor("y", [rows, F], mybir.dt.float32, kind="ExternalOutput").ap()

    RB = rows // P     # rowblocks

    with tile.TileContext(nc) as tc, ExitStack() as ctx:
        const = ctx.enter_context(tc.tile_pool(name="const", bufs=1))
        xp = ctx.enter_context(tc.tile_pool(name="xp", bufs=RB))
        xtp = ctx.enter_context(tc.tile_pool(name="xtp", bufs=RB))
        yp = ctx.enter_context(tc.tile_pool(name="yp", bufs=4))
        psp = ctx.enter_context(tc.tile_pool(name="psp", bufs=1, space="PSUM"))

        # W cast to bf16, laid out [p, k, f] with d = k*128 + p to match
        # the PE-transpose output layout of x.
        W_bf = const.tile([P, KB, F], mybir.dt.bfloat16)
        W_pkf = W.rearrange("(k p) f -> p k f", p=P)

        # Bias broadcast to all partitions, f32.
        b_bc = const.tile([P, F], mybir.dt.float32)

        # Identity for PE-based transposes; zeroed tile for clock warmup.
        ident = const.tile([P, P], mybir.dt.bfloat16)
        make_identity(nc, ident[:])
        warm = const.tile([P, P], mybir.dt.bfloat16)
        nc.any.memset(warm[:], 0.0)

        def ps0_tile():
            return psp.tile([P, NSPLIT], mybir.dt.float32, name="ps0", tag="ps0", bufs=GROUP)

        def ps1_tile():
            return psp.tile([P, NSPLIT], mybir.dt.float32, name="ps1", tag="ps1", bufs=GROUP)

        # SWDGE load order: x0-x2 and W early (k-blocks interleaved so
        # the k-outer phase can chase W's arrival), then x3..x15.
        x_tiles = [None] * RB

        def load_x(rb: int):
            x_bf = xp.tile([P, D], mybir.dt.bfloat16, name="x_bf", tag="x_bf")
            nc.gpsimd.dma_start(x_bf[:], x[rb * P:(rb + 1) * P, :])  # cast load
            x_tiles[rb] = x_bf

        load_x(0)
        nc.gpsimd.dma_start(W_bf[:, 0, :], W_pkf[:, 0, :])
        load_x(1)
        nc.gpsimd.dma_start(W_bf[:, 1, :], W_pkf[:, 1, :])
        load_x(2)
        nc.gpsimd.dma_start(W_bf[:, 2, :], W_pkf[:, 2, :])
        nc.gpsimd.dma_start(b_bc[:], b.rearrange("(o f) -> o f", o=1).to_broadcast([P, F]))
        for k in range(3, KB):
            nc.gpsimd.dma_start(W_bf[:, k, :], W_pkf[:, k, :])
        for rb in range(GROUP, RB):
            load_x(rb)

        def warmup(n):
            for _ in range(n):
                nc.tensor.matmul(
                    warm_ps[:, 0:P], warm[:], warm[:, 0:1].to_broadcast([P, P]),
                    start=True, stop=True, skip_group_check=True,
                )

        def transpose(rb: int):
            # PE transposes the 8 k-tiles into one PSUM bank; scalar and
            # vector alternate copying them back to SBUF.
            psT = psp.tile([P, KB, P], mybir.dt.bfloat16, name="psT", tag="psT", bufs=2)
            for k in range(KB):
                nc.tensor.transpose(psT[:, k, :], x_tiles[rb][:, k * P:(k + 1) * P], ident[:])
            xT = xtp.tile([P, KB, P], mybir.dt.bfloat16, name="xT", tag="xT")
            if rb % 2 == 0:
                nc.scalar.copy(xT[:], psT[:])
            else:
                nc.vector.tensor_copy(xT[:], psT[:])
            return xT

        def evict(rb: int, pss) -> None:
            y_sb = yp.tile([P, F], mybir.dt.float32)
            for n in range(NB):
                nc.vector.tensor_add(
                    y_sb[:, n * NSPLIT:(n + 1) * NSPLIT],
                    pss[n][:],
                    b_bc[:, n * NSPLIT:(n + 1) * NSPLIT],
                )
            nc.scalar.dma_start(y[rb * P:(rb + 1) * P, :], y_sb[:])

        # PE warmup ramps the clock while the first x rowblock lands;
        # more warmups pad the gaps between the early transposes.
        warm_ps = ps0_tile()
        warmup(12)

        # Phase 1 - rowblocks 0..GROUP-1: transposes as x arrives, then
        # k-outer across 6 banks following W's k-block arrivals.
        xT_tiles = {}
        for r in range(GROUP):
            xT_tiles[r] = transpose(r)
            if r < GROUP - 1:
                warmup(4)
        psA = [(ps0_tile(), ps1_tile()) for _ in range(GROUP)]
        for k in range(KB):
            for r in range(GROUP):
                for n in range(NB):
                    nc.tensor.matmul(
                        psA[r][n][:],
                        xT_tiles[r][:, k, :],
                        W_bf[:, k, n * NSPLIT:(n + 1) * NSPLIT],
                        start=(k == 0),
                        stop=(k == KB - 1),
                    )
        for r in range(GROUP):
            evict(r, psA[r])

        # Phase 2 - rowblocks GROUP..RB-1 stream with k-inner.
        for rb in range(GROUP, RB):
            xT = transpose(rb)
            pss = (ps0_tile(), ps1_tile())
            for k in range(KB):
                for n in range(NB):
                    nc.tensor.matmul(
                        pss[n][:],
                        xT[:, k, :],
                        W_bf[:, k, n * NSPLIT:(n + 1) * NSPLIT],
                        start=(k == 0),
                        stop=(k == KB - 1),
                    )
            evict(rb, pss)

    nc.compile()
    return nc


_NC_CACHE: dict[int, bass.Bass] = {}


def _get_nc(rows: int = ROWS) -> bass.Bass:
    if rows not in _NC_CACHE:
        _NC_CACHE[rows] = build_nc(rows)
    return _NC_CACHE[rows]


def _run(in_maps, rows: int = ROWS, trace: bool = False):
    nc = _get_nc(rows)
    return run_bass_kernel_spmd(nc, in_maps, list(range(N_CORES)), trace=trace)


def kernel(x: np.ndarray, W: np.ndarray, b: np.ndarray) -> np.ndarray:
    x = np.ascontiguousarray(np.asarray(x, dtype=np.float32))
    W = np.ascontiguousarray(np.asarray(W, dtype=np.float32))
    b = np.ascontiguousarray(np.asarray(b, dtype=np.float32))
    x_flat = x.reshape(ROWS_TOTAL, D)
    in_maps = [
        {"x": np.ascontiguousarray(x_flat[c * ROWS:(c + 1) * ROWS]), "W": W, "b": b}
        for c in range(N_CORES)
    ]
    res = _run(in_maps, trace=bool(int(os.environ.get("BASS_KERNEL_TRACE", "0"))))
    y = np.concatenate([res.results[c]["y"] for c in range(N_CORES)], axis=0)
    return y.reshape(B, S, F)
